# revision 16
# baseline (speedup 1.0000x reference)
"""Multi-Head Latent Attention for Trainium2, sharded over 8 NeuronCores.

Sharding: batch (2) x head-groups (4 of 4 heads each) -> 8 cores.
Each core computes, for its batch b and heads hs:
    c_Q^T = W_DQ^T x^T            (replicated within batch group)
    q_C^T, q_R^T(rope), c_KV^T, k_C^T, k_R^T(rope), v_C
    flash attention on transposed scores [t, s], causal
    partial out^T = W_O[hs]^T attnout^T
Host sums the 4 head-group partials per batch and transposes back.

All matmuls run in float32r (TF32-like, full PE rate at N=512).
Activations live transposed [features, seq]; weights are host-pretiled
into exact SBUF lhsT/rhs layouts so every DMA is wide and contiguous.
Large intermediates (c_Q^T, q_C^T, q_rope^T, c_KV^T) bounce through
DRAM scratch declared as ExternalOutput (Internal DRAM and DRAM tile
pools fail NEFF load in this environment).
"""
import numpy as np

import concourse.bass as bass
import concourse.mybir as mybir
import concourse.tile as tile
from concourse import bacc
from concourse.bass_utils import run_bass_kernel_spmd

F32 = mybir.dt.float32
F32R = mybir.dt.float32r
Exp = mybir.ActivationFunctionType.Exp
Mult = mybir.AluOpType.mult

B, S, E = 2, 2048, 2048
H = 16
DH = 128
LOW = 512
R = 64
BASE = 10000.0
HPG = 4               # heads per group (per core)
GCOL = HPG * DH       # 512 columns of this group's heads
P = 128
KE = E // P           # 16 k-tiles over E
KL = LOW // P         # 4 k-tiles over LOW
ST = S // P           # 16 seq tiles of 128
SBN = S // 512        # 4 seq blocks of 512
NEG = -3.0e38
SCALE = 1.0 / float(np.sqrt(DH + R))

_CACHE = {}


def _lhsT_layout(w):
    """[K, M] -> [MT, 128, KT, 128] so slice [mo] is an SBUF tile
    [128p, KT, 128m] with element [p, ko, m] = w[ko*128+p, mo*128+m]."""
    K, M = w.shape
    return np.ascontiguousarray(
        w.reshape(K // P, P, M // P, P).transpose(2, 1, 0, 3))


def _rhs_layout(w):
    """[K, N] -> [128, KT, N]: element [p, ko, n] = w[ko*128+p, n]."""
    K, N = w.shape
    return np.ascontiguousarray(w.reshape(K // P, P, N).transpose(1, 0, 2))


def _rope_perm_cols(w, rope_dim=R):
    """Permute each rope_dim-column block to [evens, odds] order."""
    K, M = w.shape
    nh = M // rope_dim
    w = w.reshape(K, nh, rope_dim)
    perm = np.concatenate([np.arange(0, rope_dim, 2), np.arange(1, rope_dim, 2)])
    return np.ascontiguousarray(w[:, :, perm].reshape(K, M))


def build_nc(stages=4):
    nc = bacc.Bacc("TRN2", target_bir_lowering=False, debug=False, num_devices=8)

    xT = nc.dram_tensor("xT", [E, S], F32R, kind="ExternalInput")
    wDQ = nc.dram_tensor("wDQ", [KE, P, KE, P], F32R, kind="ExternalInput")
    wUQ = nc.dram_tensor("wUQ", [HPG, P, KE, P], F32R, kind="ExternalInput")
    wQR = nc.dram_tensor("wQR", [2, P, KE, P], F32R, kind="ExternalInput")
    wDKV = nc.dram_tensor("wDKV", [KL, P, KE, P], F32R, kind="ExternalInput")
    wUK = nc.dram_tensor("wUK", [HPG, P, KL, P], F32R, kind="ExternalInput")
    wUV = nc.dram_tensor("wUV", [P, KL, GCOL], F32R, kind="ExternalInput")
    wKR = nc.dram_tensor("wKR", [P, KE, R], F32R, kind="ExternalInput")
    wO = nc.dram_tensor("wO", [P, HPG, E], F32R, kind="ExternalInput")
    cosq = nc.dram_tensor("cosq", [R, S], F32, kind="ExternalInput")   # [cos;cos]
    sinq = nc.dram_tensor("sinq", [R, S], F32, kind="ExternalInput")   # [-sin;sin]
    maskin = nc.dram_tensor("maskin", [P, 4, 512], F32, kind="ExternalInput")
    ones_in = nc.dram_tensor("ones_in", [P, 1], F32R, kind="ExternalInput")

    outT = nc.dram_tensor("outT", [E, S], F32, kind="ExternalOutput")
    # DRAM scratch (must be ExternalOutput in this environment):
    cQT = nc.dram_tensor("cQT", [E, S], F32R, kind="ExternalOutput")
    qCT = nc.dram_tensor("qCT", [HPG, P, S], F32R, kind="ExternalOutput")
    ckvS = nc.dram_tensor("ckvS", [LOW, S], F32R, kind="ExternalOutput")
    qrS = nc.dram_tensor("qrS", [HPG, R, S], F32R, kind="ExternalOutput")
    dscr = nc.dram_tensor("dscr", [SBN * HPG * 4, 512], F32, kind="ExternalOutput")

    with tile.TileContext(nc) as tc:
        with tc.tile_pool(name="persist", bufs=1) as persist:
            kropeT = persist.tile([R, S], F32R, tag="kropeT")
            t_ones = persist.tile([P, 1], F32R, tag="ones")
            nc.sync.dma_start(out=t_ones, in_=ones_in[:, :])

            with tc.tile_pool(name="tabp", bufs=1) as tabp:
                t_cos = tabp.tile([R, S], F32, tag="cos")
                t_sin = tabp.tile([R, S], F32, tag="sin")
                nc.sync.dma_start(out=t_cos, in_=cosq[:, :])
                nc.sync.dma_start(out=t_sin, in_=sinq[:, :])

                def rope_from_psum(pool, psum, base, dst_ap):
                    """dst_ap[...] (64 x 512, f32r) = rope(psum[base:base+64]).

                    psum rows [base:base+32]=x1, [base+32:base+64]=x2 (host
                    permuted weight cols). out = aln*[c;c] + swp*[-s;s],
                    written into a fresh SBUF tile then DMA'd to dst_ap."""
                    scol = dst_ap[1]
                    sl = slice(scol, scol + 512)
                    aln = pool.tile([R, 512], F32, tag="aln")
                    nc.vector.tensor_copy(out=aln, in_=psum[base:base + R, :])
                    swp = pool.tile([R, 512], F32, tag="swp")
                    nc.vector.tensor_copy(out=swp[0:32, :],
                                          in_=psum[base + 32:base + R, :])
                    nc.vector.tensor_copy(out=swp[32:R, :],
                                          in_=psum[base:base + 32, :])
                    nc.vector.tensor_mul(out=aln, in0=aln, in1=t_cos[:, sl])
                    nc.vector.tensor_mul(out=swp, in0=swp, in1=t_sin[:, sl])
                    out = pool.tile([R, 512], F32R, tag="ropeout")
                    nc.vector.tensor_add(out=out, in0=aln, in1=swp)
                    return out

                # ------- Stage 1: c_KV^T (->DRAM), k_rope^T, c_Q^T (->DRAM) -
                with (
                    tc.tile_pool(name="xt", bufs=32) as xtp,
                    tc.tile_pool(name="wdq", bufs=2) as wdqp,
                    tc.tile_pool(name="wdkv", bufs=2) as wdkvp,
                    tc.tile_pool(name="wfix1", bufs=1) as wfix1,
                    tc.tile_pool(name="s1out", bufs=2) as s1out,
                    tc.tile_pool(name="ropetmp", bufs=1) as ropetmp,
                    tc.tile_pool(name="ps_cq", bufs=4, space="PSUM") as ps_cq,
                    tc.tile_pool(name="ps_ckv", bufs=2, space="PSUM") as ps_ckv,
                    tc.tile_pool(name="ps_kr", bufs=2, space="PSUM") as ps_kr,
                ):
                    t_wkr = wfix1.tile([P, KE, R], F32R, tag="wkr")
                    nc.sync.dma_start(out=t_wkr, in_=wKR[:, :, :])

                    for half in range(2):
                        sbs = [2 * half, 2 * half + 1]
                        xts = {}
                        for sb in sbs:
                            for k in range(KE):
                                t = xtp.tile([P, 512], F32R, tag="xt")
                                nc.sync.dma_start(
                                    out=t, in_=xT[k * P:(k + 1) * P,
                                                  sb * 512:(sb + 1) * 512])
                                xts[(k, sb)] = t

                        for mo in range(KL):   # c_KV^T -> DRAM
                            t_wdkv = wdkvp.tile([P, KE, P], F32R, tag="wdkv")
                            nc.sync.dma_start(out=t_wdkv, in_=wDKV[mo])
                            for sb in sbs:
                                psum = ps_ckv.tile([P, 512], F32, tag="p")
                                for k in range(KE):
                                    nc.tensor.matmul(
                                        psum, t_wdkv[:, k, :], xts[(k, sb)],
                                        start=(k == 0), stop=(k == KE - 1))
                                ot = s1out.tile([P, 512], F32R, tag="ckvout")
                                nc.vector.tensor_copy(out=ot, in_=psum)
                                nc.sync.dma_start(
                                    out=ckvS[mo * P:(mo + 1) * P,
                                             sb * 512:(sb + 1) * 512], in_=ot)
                        for sb in sbs:         # k_rope^T (stays in SBUF)
                            psum = ps_kr.tile([R, 512], F32, tag="p")
                            for k in range(KE):
                                nc.tensor.matmul(
                                    psum, t_wkr[:, k, :], xts[(k, sb)],
                                    start=(k == 0), stop=(k == KE - 1))
                            ro = rope_from_psum(ropetmp, psum, 0, (0, sb * 512))
                            nc.vector.tensor_copy(
                                out=kropeT[:, sb * 512:(sb + 1) * 512], in_=ro)
                        for mo in range(KE):   # c_Q^T -> DRAM
                            t_wdq = wdqp.tile([P, KE, P], F32R, tag="wdq")
                            nc.sync.dma_start(out=t_wdq, in_=wDQ[mo])
                            for sb in sbs:
                                psum = ps_cq.tile([P, 512], F32, tag="p")
                                for k in range(KE):
                                    nc.tensor.matmul(
                                        psum, t_wdq[:, k, :], xts[(k, sb)],
                                        start=(k == 0), stop=(k == KE - 1))
                                ot = s1out.tile([P, 512], F32R, tag="cqout")
                                nc.vector.tensor_copy(out=ot, in_=psum)
                                nc.sync.dma_start(
                                    out=cQT[mo * P:(mo + 1) * P,
                                            sb * 512:(sb + 1) * 512], in_=ot)

                # ------- Stage 2: q_C^T (-> DRAM), q_rope^T (-> DRAM) -------
                with (
                    tc.tile_pool(name="cqt_in", bufs=20) as cqtp,
                    tc.tile_pool(name="wfix2", bufs=1) as wfix2,
                    tc.tile_pool(name="s2out", bufs=3) as s2out,
                    tc.tile_pool(name="ropetmp2", bufs=1) as ropetmp2,
                    tc.tile_pool(name="ps_qc", bufs=4, space="PSUM") as ps_qc,
                    tc.tile_pool(name="ps_qr", bufs=2, space="PSUM") as ps_qr,
                ):
                    t_wuq = [wfix2.tile([P, KE, P], F32R, tag=f"wuq{mo}",
                                        name=f"wuq{mo}") for mo in range(HPG)]
                    for mo in range(HPG):
                        nc.sync.dma_start(out=t_wuq[mo], in_=wUQ[mo])
                    t_wqr = [wfix2.tile([P, KE, P], F32R, tag=f"wqr{mo}",
                                        name=f"wqr{mo}") for mo in range(2)]
                    for mo in range(2):
                        nc.sync.dma_start(out=t_wqr[mo], in_=wQR[mo])

                    for sb in range(SBN if stages >= 2 else 0):
                        ssl = slice(sb * 512, (sb + 1) * 512)
                        cqts = []
                        for k in range(KE):
                            t = cqtp.tile([P, 512], F32R, tag="cqt")
                            nc.sync.dma_start(
                                out=t, in_=cQT[k * P:(k + 1) * P, ssl])
                            cqts.append(t)
                        for mo in range(HPG):
                            psum = ps_qc.tile([P, 512], F32, tag="p")
                            for k in range(KE):
                                nc.tensor.matmul(
                                    psum, t_wuq[mo][:, k, :], cqts[k],
                                    start=(k == 0), stop=(k == KE - 1))
                            ot = s2out.tile([P, 512], F32R, tag="qcout")
                            nc.vector.tensor_copy(out=ot, in_=psum)
                            nc.sync.dma_start(out=qCT[mo, :, ssl], in_=ot)
                        for mo in range(2):
                            psum = ps_qr.tile([P, 512], F32, tag="p")
                            for k in range(KE):
                                nc.tensor.matmul(
                                    psum, t_wqr[mo][:, k, :], cqts[k],
                                    start=(k == 0), stop=(k == KE - 1))
                            ro = rope_from_psum(ropetmp2, psum, 0, (0, sb * 512))
                            nc.sync.dma_start(out=qrS[2 * mo, :, ssl], in_=ro)
                            ro = rope_from_psum(ropetmp2, psum, R, (0, sb * 512))
                            nc.sync.dma_start(out=qrS[2 * mo + 1, :, ssl], in_=ro)

            # ------- Stage 3: k_C^T, v_C (SBUF-resident) --------------------
            kCT = persist.tile([P, HPG, S], F32R, tag="kCT")
            vC = persist.tile([P, ST, GCOL], F32R, tag="vC")
            with (
                tc.tile_pool(name="ckv_in", bufs=2) as ckvp,
                tc.tile_pool(name="wfix3", bufs=1) as wfix3,
                tc.tile_pool(name="ps_kc", bufs=3, space="PSUM") as ps_kc,
                tc.tile_pool(name="ps_vc", bufs=3, space="PSUM") as ps_vc,
            ):
                t_wuk = [wfix3.tile([P, KL, P], F32R, tag=f"wuk{mo}",
                                    name=f"wuk{mo}") for mo in range(HPG)]
                for mo in range(HPG):
                    nc.sync.dma_start(out=t_wuk[mo], in_=wUK[mo])
                t_wuv = wfix3.tile([P, KL, GCOL], F32R, tag="wuv")
                nc.sync.dma_start(out=t_wuv, in_=wUV[:, :, :])

                for sb in range(SBN if stages >= 3 else 0):
                    ssl = slice(sb * 512, (sb + 1) * 512)
                    ckvsb = ckvp.tile([P, KL, 512], F32R, tag="ckvsb")
                    for k in range(KL):
                        nc.sync.dma_start(out=ckvsb[:, k, :],
                                          in_=ckvS[k * P:(k + 1) * P, ssl])
                    for mo in range(HPG):
                        psum = ps_kc.tile([P, 512], F32, tag="p")
                        for k in range(KL):
                            nc.tensor.matmul(psum, t_wuk[mo][:, k, :],
                                             ckvsb[:, k, :],
                                             start=(k == 0), stop=(k == KL - 1))
                        nc.vector.tensor_copy(out=kCT[:, mo, ssl], in_=psum)
                    for loc in range(4):
                        st = sb * 4 + loc
                        psum = ps_vc.tile([P, GCOL], F32, tag="p")
                        for k in range(KL):
                            nc.tensor.matmul(
                                psum, ckvsb[:, k, loc * P:(loc + 1) * P],
                                t_wuv[:, k, :],
                                start=(k == 0), stop=(k == KL - 1))
                        nc.vector.tensor_copy(out=vC[:, st, :], in_=psum)

            # ------- Stage 4: attention + W_O -------------------------------
            with (
                tc.tile_pool(name="wfix4", bufs=1) as wfix4,
                tc.tile_pool(name="att", bufs=4) as att,
                tc.tile_pool(name="attsm", bufs=2) as attsm,
                tc.tile_pool(name="attq", bufs=3) as attq,
                tc.tile_pool(name="ao", bufs=2) as aop,
                tc.tile_pool(name="oout", bufs=2) as oout,
                tc.tile_pool(name="ps_s", bufs=3, space="PSUM") as ps_s,
                tc.tile_pool(name="ps_o", bufs=2, space="PSUM") as ps_o,
                tc.tile_pool(name="ps_d", bufs=1, space="PSUM") as ps_d,
                tc.tile_pool(name="ps_w", bufs=2, space="PSUM") as ps_w,
            ):
                t_wo = wfix4.tile([P, HPG, E], F32R, tag="wo")
                nc.sync.dma_start(out=t_wo, in_=wO[:, :, :])
                t_mask = wfix4.tile([P, 4, 512], F32, tag="mask")
                nc.sync.dma_start(out=t_mask, in_=maskin[:, :, :])

                for sb in range(SBN if stages >= 4 else 0):
                    aoT = aop.tile([P, HPG, 512], F32R, tag="aoT")
                    ssl = slice(sb * 512, (sb + 1) * 512)
                    for h in range(HPG):
                        qct_h = attq.tile([P, 512], F32R, tag="qct_h")
                        nc.sync.dma_start(out=qct_h, in_=qCT[h, :, ssl])
                        qrt_h = attq.tile([R, 512], F32R, tag="qrt_h")
                        nc.sync.dma_start(out=qrt_h, in_=qrS[h, :, ssl])
                        psum_o = ps_o.tile([P, 512], F32, tag="p")
                        psum_d = ps_d.tile([1, 512], F32, tag="p")
                        T = 4 * (sb + 1)
                        for tt in range(T):
                            tsl = slice(tt * P, (tt + 1) * P)
                            psum_s = ps_s.tile([P, 512], F32, tag="p")
                            nc.tensor.matmul(psum_s, kCT[:, h, tsl], qct_h,
                                             start=True, stop=False)
                            nc.tensor.matmul(psum_s, kropeT[:, tsl], qrt_h,
                                             start=False, stop=True)
                            expT = att.tile([P, 512], F32R, tag="expT")
                            if tt >= 4 * sb:
                                r = tt - 4 * sb
                                masked = attsm.tile([P, 512], F32, tag="masked")
                                nc.vector.tensor_add(out=masked, in0=psum_s,
                                                     in1=t_mask[:, r, :])
                                nc.scalar.activation(out=expT, in_=masked, func=Exp)
                            else:
                                nc.scalar.activation(out=expT, in_=psum_s, func=Exp)
                            nc.tensor.matmul(psum_d, t_ones, expT,
                                             start=(tt == 0), stop=(tt == T - 1))
                            nc.tensor.matmul(psum_o,
                                             vC[:, tt, h * DH:(h + 1) * DH], expT,
                                             start=(tt == 0), stop=(tt == T - 1))
                        # reciprocal of denominator, via DRAM-bounce broadcast
                        tmp4 = attsm.tile([4, 512], F32, tag="tmp4")
                        nc.vector.memset(tmp4, 0.0)
                        nc.vector.tensor_copy(out=tmp4[0:1, :], in_=psum_d)
                        nc.vector.reciprocal(out=tmp4[0:1, :], in_=tmp4[0:1, :])
                        drow = (sb * HPG + h) * 4
                        nc.sync.dma_start(out=dscr[drow:drow + 4, :], in_=tmp4)
                        bc = attsm.tile([P, 512], F32, tag="bc")
                        src = dscr[drow, :]
                        nc.gpsimd.dma_start(out=bc, in_=bass.AP(
                            tensor=src.tensor, offset=src.offset,
                            ap=[[0, P]] + [list(x) for x in src.ap]))
                        nc.vector.tensor_tensor(aoT[:, h, :], psum_o, bc, Mult)
                    # W_O partial for this s-block
                    for mo in range(KE):
                        psum_w = ps_w.tile([P, 512], F32, tag="p")
                        for k in range(HPG):
                            nc.tensor.matmul(psum_w,
                                             t_wo[:, k, mo * P:(mo + 1) * P],
                                             aoT[:, k, :],
                                             start=(k == 0), stop=(k == HPG - 1))
                        ot = oout.tile([P, 512], F32, tag="oout")
                        nc.vector.tensor_copy(out=ot, in_=psum_w)
                        nc.sync.dma_start(out=outT[mo * P:(mo + 1) * P, ssl], in_=ot)

    nc.compile()
    return nc


def _host_inputs(inputs):
    """Per-core input maps (host-side sharding + weight pre-tiling)."""
    x = inputs["x"]
    W_DQ, W_UQ, W_QR = inputs["W_DQ"], inputs["W_UQ"], inputs["W_QR"]
    W_DKV, W_UK, W_KR = inputs["W_DKV"], inputs["W_UK"], inputs["W_KR"]
    W_UV, W_O = inputs["W_UV"], inputs["W_O"]

    # shared across cores
    wDQ_t = _lhsT_layout(W_DQ.astype(np.float32))
    wDKV_t = _lhsT_layout(W_DKV.astype(np.float32))
    wKR_t = _rhs_layout(_rope_perm_cols(W_KR.astype(np.float32)))  # [128,KE,64]
    half = R // 2
    freqs = BASE ** (-np.arange(half, dtype=np.float64) / half)
    theta = np.arange(S, dtype=np.float64)[None, :] * freqs[:, None]   # [32, S]
    cos2 = np.concatenate([np.cos(theta), np.cos(theta)], 0).astype(np.float32)
    sinpm = np.concatenate([-np.sin(theta), np.sin(theta)], 0).astype(np.float32)
    p = np.arange(P)[:, None, None]
    rr = np.arange(4)[None, :, None]
    f = np.arange(512)[None, None, :]
    maskadd = np.where(p <= f - P * rr, 0.0, NEG).astype(np.float32)
    ones = np.ones((P, 1), np.float32)

    in_maps = []
    for c in range(8):
        b, g = divmod(c, 4)
        cs, ce = g * GCOL, (g + 1) * GCOL          # head cols of this group
        wUQ_g = _lhsT_layout(W_UQ[:, cs:ce].astype(np.float32) * SCALE)
        qr = W_QR[:, g * HPG * R:(g + 1) * HPG * R].astype(np.float32) * SCALE
        wQR_g = _lhsT_layout(_rope_perm_cols(qr))
        wUK_g = _lhsT_layout(W_UK[:, cs:ce].astype(np.float32))
        wUV_g = _rhs_layout(W_UV[:, cs:ce].astype(np.float32))
        wO_g = _rhs_layout(W_O[cs:ce, :].astype(np.float32))       # [128, 4, E]
        in_maps.append({
            "xT": np.ascontiguousarray(x[b].T),
            "wDQ": wDQ_t, "wUQ": wUQ_g, "wQR": wQR_g, "wDKV": wDKV_t,
            "wUK": wUK_g, "wUV": wUV_g, "wKR": wKR_t, "wO": wO_g,
            "cosq": cos2, "sinq": sinpm, "maskin": maskadd, "ones_in": ones,
        })
    return in_maps


def _assemble(results):
    out = np.empty((B, S, E), np.float32)
    for b in range(B):
        acc = results[4 * b]["outT"].astype(np.float32).copy()
        for g in range(1, 4):
            acc += results[4 * b + g]["outT"]
        out[b] = acc.T
    return out


def kernel(**inputs):
    inputs = {k: np.asarray(v) for k, v in inputs.items()}
    if "nc" not in _CACHE:
        _CACHE["nc"] = build_nc()
    nc = _CACHE["nc"]
    in_maps = _host_inputs(inputs)
    res = run_bass_kernel_spmd(nc, in_maps, core_ids=list(range(8)))
    return _assemble(res.results)
